# revision 23
# baseline (speedup 1.0000x reference)
"""Trainium2 Bass kernel for NRDF adapter (29-joint BoneMLP tree + DFNet).

Data parallel over 8 cores (16384 samples each), activations feature-major
([features, batch]) so every matmul streams 512-col batch blocks against
stationary host-prepped bf16 weights (fp32 PSUM accumulate).

Design notes (v3):
- bf16 weights + activations: ~5e-3 rel err on host sim (gate 2e-2).
- Exact softplus(beta=100), storage s = 100*softplus_b(z), PSUM P = 50*z:
    variant A (DVE-heavy):  r = max(2P,0)   [DVE ts, psum]
                            q = P - r       [DVE stt, psum]  (2q = -|100z|)
                            e = exp(2q)     [ACT]
                            c = ln(e+1)     [ACT]
                            s = r + c       [DVE tt, bf16 SBUF = 2x/4x mode]
    variant B (ACT-heavy):  m = |2P|        [ACT abs, psum]
                            e = exp(-m)     [ACT]
                            c = ln(e+1)     [ACT]
                            r = 0.5m + P    [DVE stt, psum]
                            s = r + c       [DVE tt]
  The A/B mix balances DVE vs ACT engine time.  No GpSimd anywhere.
- All biases ride ones-rows inside existing matmuls (xf row 29, h row 68,
  bin0 row 112) except DFNet L1/L2 which use K=1 ones matmuls.
- Abs/Exp/Ln pinned to the natural_log_exp_and_others ACT table set ->
  zero table reloads.
- Program order interleaves pairs of 1024-col batch units so each engine's
  in-order queue always holds work from two independent dependency chains.
- Tree levels place their 16G-row latent blocks directly into the 4
  DFNet-contraction bins (softplus writes at the 32-aligned offset); the
  next level's parent-feature matmul reads the bin from partition 0 with
  zero-padded weight rows (matmul cost is column-dominated, so free).
"""

import numpy as np
from contextlib import ExitStack

import ml_dtypes

import concourse.bass as bass
import concourse.mybir as mybir
import concourse.hw_specs as hw_specs
from concourse import bacc
from concourse.tile import TileContext
from concourse.bass_utils import run_bass_kernel_spmd

BF16_NP = ml_dtypes.bfloat16


class _Bacc(bacc.Bacc):
    """Pin Abs/Exp/Ln to the combined natural_log_exp set so the ACT engine
    never reloads tables."""

    def insert_act_table_loads(self):
        has_activation = any(
            isinstance(i, mybir.InstActivation)
            for b in self.main_func.blocks
            for i in b.instructions)
        if not has_activation:
            return
        tables = list(hw_specs.get_activation_tables(self.m.arch).items())
        pinned = (EXP, LN, ABS, RELU)
        tables = [
            (name,
             fns if name == "natural_log_exp_and_others" else
             {f for f in fns if f not in pinned})
            for name, fns in tables
        ]
        bacc._bass_rust.insert_act_table_loads(self, tables)


F32 = mybir.dt.float32
BF16 = mybir.dt.bfloat16
EXP = mybir.ActivationFunctionType.Exp
LN = mybir.ActivationFunctionType.Ln
ABS = mybir.ActivationFunctionType.Abs
RELU = mybir.ActivationFunctionType.Relu
COPY = mybir.ActivationFunctionType.Copy
ALU = mybir.AluOpType

N_CORES = 8
B_FULL = 131072
B_CORE = B_FULL // N_CORES
J, F, H = 29, 16, 17
PARENT = [12, 0, 1, 2, 3, 4, 12, 6, 7, 8, 9, 10, -1, 12, 13, 14, 15, 16, 17,
          18, 19, 20, 14, 22, 23, 24, 25, 26, 27]


def _levels():
    def depth(i):
        d = 0
        while PARENT[i] != -1:
            i = PARENT[i]
            d += 1
        return d
    by_d = {}
    for i in range(J):
        by_d.setdefault(depth(i), []).append(i)
    return [sorted(by_d[k]) for k in range(len(by_d))]


LEVELS = _levels()
NL = len(LEVELS)
NG = [len(l) for l in LEVELS]
M1S = [17 * g for g in NG]
M2S = [16 * g for g in NG]
# level -> (bin index, partition offset) of its 16G-row latent block;
# offsets 32-aligned (hardware partition-base requirement).
PLACE = {1: (0, 0), 2: (0, 64), 3: (1, 0), 4: (1, 64), 5: (2, 0), 6: (2, 64),
         7: (3, 0), 8: (3, 32), 9: (3, 64), 0: (3, 96)}
BIN_K = [113, 128, 128, 112]     # contraction depth (bin0 incl bias row 112)
ONES_H = 68                      # ones row in the h tile (layer2 bias)
ONES_BIN0 = 112                  # ones row in bin0 (DFNet L0 bias)

# softplus calls computed ACT-heavy (variant B) to balance DVE vs ACT time
B_CALLS = {("f", 6), ("d0", 0), ("d0", 1), ("d0", 2), ("d0", 3)}

for _l in range(1, NL):
    for _j in LEVELS[_l]:
        assert PARENT[_j] in LEVELS[_l - 1]


def _bone_layout():
    off = {}
    c = 0
    for l in range(NL):
        off[f"B{l}"] = c; c += M1S[l]     # rows 0:29 x-weights + row 29 bias
    for l in range(1, NL):
        off[f"A{l}"] = c; c += M1S[l]     # parent-feat weights at parent rows
    for l in range(NL):
        off[f"C{l}"] = c; c += M2S[l]     # rows 0:M1 W2^T/2 + row 68 bias
    return off, c


def _wd_layout():
    off = {}
    c = 0
    off["wd0"] = c; c += 4 * 512     # per-bin lhsT chunks (row 112 of b0: bias)
    off["wd1"] = c; c += 4 * 256
    off["wd2"] = c; c += 2 * 128
    off["wd3"] = c; c += 1
    off["bd1"] = c; c += 2           # columns: 100*bd1 per mc chunk
    off["bd2"] = c; c += 1           # column: 100*bd2
    return off, c


BONE_OFF, CB = _bone_layout()
WD_OFF, CW = _wd_layout()


def prep_weights(W1, b1, W2, b2, Wd0, bd0, Wd1, bd1, Wd2, bd2, Wd3, bd3):
    bone = np.zeros((128, CB), np.float32)
    for l, joints in enumerate(LEVELS):
        B_off = BONE_OFF[f"B{l}"]
        C_off = BONE_OFF[f"C{l}"]
        for g, j in enumerate(joints):
            cols = slice(B_off + g * 17, B_off + (g + 1) * 17)
            bone[j, cols] = 50.0 * W1[j][:, 0]
            bone[29, cols] = 50.0 * b1[j]
            bone[g * 17:(g + 1) * 17,
                 C_off + g * 16: C_off + (g + 1) * 16] = W2[j].T / 2.0
            bone[ONES_H,
                 C_off + g * 16: C_off + (g + 1) * 16] = 50.0 * b2[j]
        if l > 0:
            A_off = BONE_OFF[f"A{l}"]
            prev = LEVELS[l - 1]
            pr0 = PLACE[l - 1][1]   # A rows co-located with parent bin rows
            for g, j in enumerate(joints):
                q = prev.index(PARENT[j])
                bone[pr0 + q * 16:pr0 + (q + 1) * 16,
                     A_off + g * 17: A_off + (g + 1) * 17] = W1[j][:, 1:].T / 2.0

    wd = np.zeros((128, CW), np.float32)
    for l, joints in enumerate(LEVELS):
        bi, r0 = PLACE[l]
        for g, j in enumerate(joints):
            wd[r0 + g * 16: r0 + (g + 1) * 16,
               WD_OFF["wd0"] + bi * 512: WD_OFF["wd0"] + (bi + 1) * 512] = \
                Wd0[:, j * 16:(j + 1) * 16].T / 2.0
    wd[ONES_BIN0, WD_OFF["wd0"]:WD_OFF["wd0"] + 512] = 50.0 * bd0
    for kc in range(4):
        wd[:, WD_OFF["wd1"] + kc * 256: WD_OFF["wd1"] + (kc + 1) * 256] = \
            Wd1[:, kc * 128:(kc + 1) * 128].T / 2.0
    for kc in range(2):
        wd[:, WD_OFF["wd2"] + kc * 128: WD_OFF["wd2"] + (kc + 1) * 128] = \
            Wd2[:, kc * 128:(kc + 1) * 128].T / 2.0
    wd[:, WD_OFF["wd3"]] = Wd3[0, :] / 2.0
    for mc in range(2):
        wd[:, WD_OFF["bd1"] + mc] = 100.0 * bd1[mc * 128:(mc + 1) * 128]
    wd[:, WD_OFF["bd2"]] = 100.0 * bd2
    return bone.astype(BF16_NP), wd.astype(BF16_NP)


def build_nc(b_core=B_CORE, n_cores=N_CORES):
    T = b_core // 512
    NP = b_core // 1024
    assert NP % 2 == 0
    nc = _Bacc("TRN2", target_bir_lowering=False, debug=False,
               num_devices=n_cores)
    xt_d = nc.dram_tensor("xt", [32, b_core], BF16, kind="ExternalInput")
    bone_d = nc.dram_tensor("bone", [128, CB], BF16, kind="ExternalInput")
    wd_d = nc.dram_tensor("wd", [128, CW], BF16, kind="ExternalInput")
    ones_d = nc.dram_tensor("ones_v", [1, 1024], BF16, kind="ExternalInput")
    y_d = nc.dram_tensor("y", [b_core], F32, kind="ExternalOutput")

    with ExitStack() as ctx:
        tc = ctx.enter_context(TileContext(nc))
        wp = ctx.enter_context(tc.tile_pool(name="w", bufs=1))
        psp = ctx.enter_context(tc.tile_pool(name="ps", bufs=4, space="PSUM"))
        xfp = ctx.enter_context(tc.tile_pool(name="xfp", bufs=1))
        sgp = ctx.enter_context(tc.tile_pool(name="sgp", bufs=5))
        otp = ctx.enter_context(tc.tile_pool(name="otp", bufs=2))

        bone = wp.tile([128, CB], BF16, name="bone_sb")
        nc.sync.dma_start(out=bone[:, :], in_=bone_d[:, :])
        wdt = wp.tile([128, CW], BF16, name="wd_sb")
        nc.sync.dma_start(out=wdt[:, :], in_=wd_d[:, :])
        xfall = wp.tile([32, b_core], BF16, name="x_all")
        ch = b_core // 4
        for c0 in range(0, b_core, ch):
            nc.sync.dma_start(out=xfall[:, c0:c0 + ch],
                              in_=xt_d[:, c0:c0 + ch])

        # persistent double-buffered activation tiles (manual u%2 alternation)
        NB = 8   # buffer sets; four unit-pairs in flight on disjoint sets
        ht = [xfp.tile([128, 1024], BF16, name=f"ht{b}") for b in range(NB)]
        bins = [[xfp.tile([128, 1024], BF16, name=f"bin{i}_{b}")
                 for b in range(NB)] for i in range(4)]
        # DFNet hidden tiles rotate in a shared pool (3 units in flight);
        # tile objects are remembered per unit so df1/df2/df3 read the
        # right acquisition
        dfp = ctx.enter_context(tc.tile_pool(name="dfp", bufs=2))
        h1 = {}
        h2 = {}
        h3 = {}

        # presets: zero everything contracted with garbage rows; ones rows
        # land at unaligned partitions, so they go in by DMA.
        for b in range(NB):
            nc.vector.memset(ht[b][:, :], 0.0)
            for i in range(4):
                nc.vector.memset(bins[i][b][:, :], 0.0)
            nc.sync.dma_start(out=ht[b][ONES_H:ONES_H + 1, :], in_=ones_d[:, :])
            nc.sync.dma_start(out=bins[0][b][ONES_BIN0:ONES_BIN0 + 1, :],
                              in_=ones_d[:, :])

        def softplus(P, M, dst, key, nm):
            """dst[0:M] = 100*softplus_b(P/50), exact, bf16."""
            r = sgp.tile([128, 1024], BF16, tag="r", name=f"r{nm}")
            c = sgp.tile([128, 1024], BF16, tag="c", name=f"c{nm}")
            e = sgp.tile([128, 1024], BF16, tag="e", name=f"e{nm}")
            if key in B_CALLS:
                m = sgp.tile([128, 1024], BF16, tag="m", name=f"m{nm}")
                nc.scalar.activation(m[0:M, :], P, ABS, scale=2.0)
                nc.scalar.activation(e[0:M, :], m[0:M, :], EXP, scale=-1.0)
                nc.scalar.activation(c[0:M, :], e[0:M, :], LN, bias=1.0)
                nc.vector.scalar_tensor_tensor(r[0:M, :], m[0:M, :], 0.5, P,
                                               op0=ALU.mult, op1=ALU.add)
            else:
                nc.vector.tensor_scalar(r[0:M, :], P, 2.0, 0.0,
                                        op0=ALU.mult, op1=ALU.max)
                q = sgp.tile([128, 1024], BF16, tag="q", name=f"q{nm}")
                nc.vector.scalar_tensor_tensor(q[0:M, :], r[0:M, :], -1.0, P,
                                               op0=ALU.mult, op1=ALU.add)
                nc.scalar.activation(e[0:M, :], q[0:M, :], EXP, scale=2.0)
                nc.scalar.activation(c[0:M, :], e[0:M, :], LN, bias=1.0)
            nc.vector.tensor_tensor(dst, r[0:M, :], c[0:M, :], op=ALU.add)

        def softplus_biased(P, bias, dst, nm):
            """dst = 100*softplus_b((P + 50*bias')/50) with bias' folded via
            the ACT bias port (bias holds 100*bd); all table ops, one add."""
            r = sgp.tile([128, 1024], BF16, tag="r", name=f"r{nm}")
            c = sgp.tile([128, 1024], BF16, tag="c", name=f"c{nm}")
            e = sgp.tile([128, 1024], BF16, tag="e", name=f"e{nm}")
            m = sgp.tile([128, 1024], BF16, tag="m", name=f"m{nm}")
            nc.scalar.activation(m[:, :], P, ABS, scale=2.0, bias=bias)
            nc.scalar.activation(e[:, :], m[:, :], EXP, scale=-1.0)
            nc.scalar.activation(c[:, :], e[:, :], LN, bias=1.0)
            nc.scalar.activation(r[:, :], P, RELU, scale=2.0, bias=bias)
            nc.vector.tensor_tensor(dst, r[:, :], c[:, :], op=ALU.add)

        def emit_layer1(u, l):
            ub = u % NB
            M1 = M1S[l]
            ph = psp.tile([128, 1024], F32, tag="ps", name=f"ph{u}_{l}")
            if l > 0:
                pbi, pr0 = PLACE[l - 1]
                Kp = pr0 + M2S[l - 1]   # read bin from row 0; rows 0:pr0 hit
                a0 = BONE_OFF[f"A{l}"]  # zero weights (cost is N-dominated)
                for hh in range(2):
                    s_ = slice(hh * 512, (hh + 1) * 512)
                    nc.tensor.matmul(
                        ph[0:M1, s_], bone[0:Kp, a0:a0 + M1],
                        bins[pbi][ub][0:Kp, s_],
                        start=True, stop=False, skip_group_check=True)
            b0 = BONE_OFF[f"B{l}"]
            for hh in range(2):
                s_ = slice(hh * 512, (hh + 1) * 512)
                nc.tensor.matmul(ph[0:M1, s_], bone[0:30, b0:b0 + M1],
                                 xfall[0:30, u * 1024 + hh * 512:
                                       u * 1024 + (hh + 1) * 512],
                                 start=(l == 0), stop=True,
                                 skip_group_check=True)
            softplus(ph[0:M1, :], M1, ht[ub][0:M1, :], ("h", l), f"h{u}_{l}")

        def emit_layer1_pair(up, l):
            # both units' layer-1 preacts in ONE PSUM tile (unit up at rows
            # 0:M1, up+1 at 64:64+M1 via tile_position col offset); one
            # softplus pass, per-unit adds into each unit's ht.
            M1 = M1S[l]
            ph = psp.tile([128, 1024], F32, tag="ps", name=f"php{up}_{l}")
            for u in (up, up + 1):
                ub = u % NB
                o = 64 * (u % 2)
                if l > 0:
                    pbi, pr0 = PLACE[l - 1]
                    Kp = pr0 + M2S[l - 1]
                    a0 = BONE_OFF[f"A{l}"]
                    for hh in range(2):
                        s_ = slice(hh * 512, (hh + 1) * 512)
                        nc.tensor.matmul(
                            ph[o:o + M1, s_], bone[0:Kp, a0:a0 + M1],
                            bins[pbi][ub][0:Kp, s_],
                            start=True, stop=False, skip_group_check=True,
                            tile_position=(0, o))
                b0 = BONE_OFF[f"B{l}"]
                for hh in range(2):
                    s_ = slice(hh * 512, (hh + 1) * 512)
                    nc.tensor.matmul(ph[o:o + M1, s_], bone[0:30, b0:b0 + M1],
                                     xfall[0:30, u * 1024 + hh * 512:
                                           u * 1024 + (hh + 1) * 512],
                                     start=(l == 0), stop=True,
                                     skip_group_check=True,
                                     tile_position=(0, o))
            MP = 64 + M1
            nm = f"h{up}_{l}"
            r = sgp.tile([128, 1024], BF16, tag="r", name=f"r{nm}")
            c = sgp.tile([128, 1024], BF16, tag="c", name=f"c{nm}")
            e = sgp.tile([128, 1024], BF16, tag="e", name=f"e{nm}")
            P = ph[0:MP, :]
            if ("h", l) in B_CALLS:
                m = sgp.tile([128, 1024], BF16, tag="m", name=f"m{nm}")
                nc.scalar.activation(m[0:MP, :], P, ABS, scale=2.0)
                nc.scalar.activation(e[0:MP, :], m[0:MP, :], EXP, scale=-1.0)
                nc.scalar.activation(c[0:MP, :], e[0:MP, :], LN, bias=1.0)
                nc.vector.scalar_tensor_tensor(r[0:MP, :], m[0:MP, :], 0.5, P,
                                               op0=ALU.mult, op1=ALU.add)
            else:
                nc.vector.tensor_scalar(r[0:MP, :], P, 2.0, 0.0,
                                        op0=ALU.mult, op1=ALU.max)
                q = sgp.tile([128, 1024], BF16, tag="q", name=f"q{nm}")
                nc.vector.scalar_tensor_tensor(q[0:MP, :], r[0:MP, :], -1.0, P,
                                               op0=ALU.mult, op1=ALU.add)
                nc.scalar.activation(e[0:MP, :], q[0:MP, :], EXP, scale=2.0)
                nc.scalar.activation(c[0:MP, :], e[0:MP, :], LN, bias=1.0)
            for u in (up, up + 1):
                ub = u % NB
                o = 64 * (u % 2)
                nc.vector.tensor_tensor(ht[ub][0:M1, :],
                                        r[o:o + M1, :], c[o:o + M1, :],
                                        op=ALU.add)

        def emit_l1_stage(up, l):
            # layer 1 pair-packed when M1 fits under the 64-row col offset
            if M1S[l] <= 64:
                emit_layer1_pair(up, l)
            else:
                emit_layer1(up, l)
                emit_layer1(up + 1, l)

        def emit_l2_stage(up, l):
            # layer 2 always pair-packs (M2 <= 64)
            M1, M2 = M1S[l], M2S[l]
            pf = psp.tile([128, 1024], F32, tag="ps", name=f"pf{up}_{l}")
            cc = BONE_OFF[f"C{l}"]
            for u in (up, up + 1):
                ub = u % NB
                o = 64 * (u % 2)
                for hh in range(2):
                    s_ = slice(hh * 512, (hh + 1) * 512)
                    nc.tensor.matmul(pf[o:o + M2, s_],
                                     bone[0:ONES_H + 1, cc:cc + M2],
                                     ht[ub][0:ONES_H + 1, s_],
                                     start=True, stop=True,
                                     skip_group_check=True,
                                     tile_position=(0, o))
            bi, r0 = PLACE[l]
            MP = 64 + M2
            nm = f"f{up}_{l}"
            r = sgp.tile([128, 1024], BF16, tag="r", name=f"r{nm}")
            c = sgp.tile([128, 1024], BF16, tag="c", name=f"c{nm}")
            e = sgp.tile([128, 1024], BF16, tag="e", name=f"e{nm}")
            P = pf[0:MP, :]
            if ("f", l) in B_CALLS:
                m = sgp.tile([128, 1024], BF16, tag="m", name=f"m{nm}")
                nc.scalar.activation(m[0:MP, :], P, ABS, scale=2.0)
                nc.scalar.activation(e[0:MP, :], m[0:MP, :], EXP, scale=-1.0)
                nc.scalar.activation(c[0:MP, :], e[0:MP, :], LN, bias=1.0)
                nc.vector.scalar_tensor_tensor(r[0:MP, :], m[0:MP, :], 0.5, P,
                                               op0=ALU.mult, op1=ALU.add)
            else:
                nc.vector.tensor_scalar(r[0:MP, :], P, 2.0, 0.0,
                                        op0=ALU.mult, op1=ALU.max)
                q = sgp.tile([128, 1024], BF16, tag="q", name=f"q{nm}")
                nc.vector.scalar_tensor_tensor(q[0:MP, :], r[0:MP, :], -1.0, P,
                                               op0=ALU.mult, op1=ALU.add)
                nc.scalar.activation(e[0:MP, :], q[0:MP, :], EXP, scale=2.0)
                nc.scalar.activation(c[0:MP, :], e[0:MP, :], LN, bias=1.0)
            for u in (up, up + 1):
                ub = u % NB
                o = 64 * (u % 2)
                nc.vector.tensor_tensor(bins[bi][ub][r0:r0 + M2, :],
                                        r[o:o + M2, :], c[o:o + M2, :],
                                        op=ALU.add)

        def emit_df0(u, mc):
            ub = u % NB
            p0 = psp.tile([128, 1024], F32, tag="ps", name=f"p0_{u}_{mc}")
            for kc in range(4):
                w0 = WD_OFF["wd0"] + kc * 512 + mc * 128
                for hh in range(2):
                    s_ = slice(hh * 512, (hh + 1) * 512)
                    nc.tensor.matmul(p0[:, s_], wdt[0:BIN_K[kc], w0:w0 + 128],
                                     bins[kc][ub][0:BIN_K[kc], s_],
                                     start=(kc == 0), stop=(kc == 3),
                                     skip_group_check=True)
            if mc == 0:
                h1[u] = {}
            h1[u][mc] = dfp.tile([128, 1024], BF16, tag=f"h1_{mc}",
                                 name=f"h1_{mc}_{u}")
            softplus(p0[:, :], 128, h1[u][mc][:, :], ("d0", mc),
                     f"d0_{u}_{mc}")

        def emit_df1(u, mc):
            ub = u % NB
            p1 = psp.tile([128, 1024], F32, tag="ps", name=f"p1_{u}_{mc}")
            for kc in range(4):
                w1 = WD_OFF["wd1"] + kc * 256 + mc * 128
                for hh in range(2):
                    s_ = slice(hh * 512, (hh + 1) * 512)
                    nc.tensor.matmul(p1[:, s_], wdt[0:128, w1:w1 + 128],
                                     h1[u][kc][:, s_],
                                     start=(kc == 0), stop=(kc == 3),
                                     skip_group_check=True)
            bb = WD_OFF["bd1"] + mc
            if mc == 0:
                h2[u] = {}
            h2[u][mc] = dfp.tile([128, 1024], BF16, tag=f"h2_{mc}",
                                 name=f"h2_{mc}_{u}")
            softplus_biased(p1[:, :], wdt[0:128, bb:bb + 1],
                            h2[u][mc][:, :], f"d1_{u}_{mc}")

        def emit_df2(u):
            ub = u % NB
            p2 = psp.tile([128, 1024], F32, tag="ps", name=f"p2_{u}")
            for kc in range(2):
                w2 = WD_OFF["wd2"] + kc * 128
                for hh in range(2):
                    s_ = slice(hh * 512, (hh + 1) * 512)
                    nc.tensor.matmul(p2[:, s_], wdt[0:128, w2:w2 + 128],
                                     h2[u][kc][:, s_],
                                     start=(kc == 0), stop=(kc == 1),
                                     skip_group_check=True)
            h3[u] = dfp.tile([128, 1024], BF16, tag="h3", name=f"h3_{u}")
            softplus_biased(p2[:, :], wdt[0:128, WD_OFF["bd2"]:
                                          WD_OFF["bd2"] + 1],
                            h3[u][:, :], f"d2_{u}")

        def emit_df3(u):
            ub = u % NB
            pd = psp.tile([128, 1024], F32, tag="ps", name=f"pd{u}")
            w3 = WD_OFF["wd3"]
            for hh in range(2):
                s_ = slice(hh * 512, (hh + 1) * 512)
                nc.tensor.matmul(pd[0:1, s_], wdt[0:128, w3:w3 + 1],
                                 h3[u][:, s_],
                                 start=True, stop=True, skip_group_check=True)
            ot = otp.tile([1, 1024], F32, tag="ot", name=f"ot{u}")
            nc.scalar.activation(ot[0:1, :], pd[0:1, :], COPY)
            # raw 50*z3 (unbiased); host adds bd3 + softplus
            dst = bass.AP(y_d, u * 1024, [[1024, 1], [1, 1024]])
            nc.sync.dma_start(out=dst, in_=ot[0:1, :])

        # interleave TWO pairs (a quad of units on disjoint buffer sets) so
        # the 4-tile PSUM rotation always spans two independent dependency
        # chains -- emitting pairs back-to-back would serialize them on PSUM
        per_unit = ([lambda u, m=m: emit_df0(u, m) for m in range(4)]
                    + [lambda u, m=m: emit_df1(u, m) for m in range(2)]
                    + [emit_df2, emit_df3])
        assert NP % 8 == 0
        for g0 in range(0, NP, 8):
            pairs = (g0, g0 + 2, g0 + 4, g0 + 6)
            for l in range(NL):
                for p in pairs:
                    emit_l1_stage(p, l)
                for p in pairs:
                    emit_l2_stage(p, l)
            for u in range(g0, g0 + 8):
                for stage in per_unit:
                    stage(u)
    nc.compile()
    return nc


_NC_CACHE = {}


def _get_nc(b_core):
    if b_core not in _NC_CACHE:
        _NC_CACHE[b_core] = build_nc(b_core)
    return _NC_CACHE[b_core]


def kernel(x, W1, b1, W2, b2, Wd0, bd0, Wd1, bd1, Wd2, bd2, Wd3, bd3,
           _trace=False):
    x = np.ascontiguousarray(np.asarray(x, dtype=np.float32))
    B = x.shape[0]
    assert B % N_CORES == 0
    b_core = B // N_CORES
    args = [np.asarray(a, dtype=np.float32) for a in
            (W1, b1, W2, b2, Wd0, bd0, Wd1, bd1, Wd2, bd2, Wd3, bd3)]
    bone, wd = prep_weights(*args)
    nc = _get_nc(b_core)
    ones_v = np.ones((1, 1024), BF16_NP)
    xt_full = np.zeros((N_CORES, 32, b_core), BF16_NP)
    xb = x.astype(BF16_NP)
    for c in range(N_CORES):
        xt_full[c, 0:29] = xb[c * b_core:(c + 1) * b_core].T
    xt_full[:, 29] = 1.0
    in_maps = [{"xt": xt_full[c], "bone": bone, "wd": wd, "ones_v": ones_v}
               for c in range(N_CORES)]
    res = run_bass_kernel_spmd(nc, in_maps, list(range(N_CORES)), trace=_trace)
    p_out = np.concatenate([res.results[c]["y"] for c in range(N_CORES)])
    kernel.last_result = res
    # z3 = P/50 + bd3; final softplus on host (exact, float64)
    t = (p_out.astype(np.float64) / 50.0
         + float(np.asarray(bd3, np.float64)[0])) * 100.0
    out = np.logaddexp(t, 0.0) / 100.0
    return out.astype(np.float32)


kernel.last_result = None
